# revision 8
# baseline (speedup 1.0000x reference)
"""SimCLR contrastive-loss kernel for 8 Trainium2 NeuronCores.

Full inputs in, full outputs out.  No collectives: proj_2 is replicated
to every core (host-side slicing/replication only); each core normalizes
all of z2 redundantly and computes its own 1024-row block of the
similarity matrix.  Matmul data is bf16; transposes run on the DMA XBAR
(dma_start_transpose), so the PE does only the 256 main matmuls and the
PSUM is wholly owned by the main loop (4 rotating [128,1024] f32 tiles =
8 banks).  The x side is scaled by -1000/||x|| so PSUM holds -1000*sim
and the DVE reduce_min output is directly the exp bias; ACT exp runs
in-place on PSUM with accumulate.  Exact per-group logsumexp fixup.
Positives are computed exactly in f32 from the raw shards.

Engine budget per column group (8 groups x 8 row-tiles):
  PE   4 matmuls x 512 rows        DVE  reduce_min + y-square reduce
  ACT  exp+accum + y-square accum  GPSIMD  y scale+cast to bf16
"""

import numpy as np

B = 8192          # batch
D = 256           # feature dim
NCORES = 8
R = B // NCORES   # rows per core = 1024
P = 128           # partitions
MT = R // P       # x tiles per core = 8
YT = B // P       # y tiles per core = 64
GROUP = 1024      # columns per logsumexp group
NG = B // GROUP   # groups per row = 8
CHT = 8           # y tiles per chunk (= one group of columns)
NS = 512          # matmul moving free dim (one PSUM bank)
TEMP_INV = 1000.0
LN_TEMP_INV = float(np.log(1000.0))

_CACHE = {}


def _build_nc():
    import concourse.bacc as bacc
    import concourse.mybir as mybir
    from concourse import tile

    f32 = mybir.dt.float32
    bf16 = mybir.dt.bfloat16
    AOT = mybir.AluOpType
    ACT = mybir.ActivationFunctionType
    AXL = mybir.AxisListType

    nc = bacc.Bacc("TRN2", target_bir_lowering=False, debug=False,
                   num_devices=NCORES)

    p1 = nc.dram_tensor("p1", [R, D], f32, kind="ExternalInput")
    p2 = nc.dram_tensor("p2", [B, D], f32, kind="ExternalInput")
    p2s = nc.dram_tensor("p2s", [R, D], f32, kind="ExternalInput")
    res = nc.dram_tensor("res", [P, 2 * MT], f32, kind="ExternalOutput")

    with tile.TileContext(nc) as tc:
        with (
            tc.tile_pool(name="big", bufs=1) as big,
            tc.tile_pool(name="yin", bufs=16) as yin,
            tc.tile_pool(name="scr", bufs=4) as scr,
        ):
            # persistent SBUF tensors
            z2T0 = big.tile([P, B], bf16, tag="z2T0")   # z2^T dims 0..127
            z2T1 = big.tile([P, B], bf16, tag="z2T1")   # z2^T dims 128..255
            xT0 = big.tile([P, R], bf16, tag="xT0")     # (-1000*x^)^T d lo
            xT1 = big.tile([P, R], bf16, tag="xT1")
            xs = big.tile([P, MT * D], f32, tag="xs")   # p1 shard natural
            ys2 = big.tile([P, MT * D], f32, tag="ys2")  # p2 shard natural
            xsb0 = big.tile([P, MT, P], bf16, tag="xsb0")  # scaled x, d lo
            xsb1 = big.tile([P, MT, P], bf16, tag="xsb1")  # scaled x, d hi
            ysb0 = big.tile([P, CHT, P], bf16, tag="ysb0")  # scaled y chunk
            ysb1 = big.tile([P, CHT, P], bf16, tag="ysb1")
            n2x = big.tile([P, MT], f32, tag="n2x")
            n2y = big.tile([P, YT], f32, tag="n2y")
            n2o = big.tile([P, MT], f32, tag="n2o")     # own p2 shard norms
            rix = big.tile([P, MT], f32, tag="rix")     # -1000*rsqrt(n2x)
            riy = big.tile([P, YT], f32, tag="riy")     # rsqrt(n2y)
            rio = big.tile([P, MT], f32, tag="rio")     # rsqrt(n2o)
            tln = big.tile([P, YT], f32, tag="tln")
            praw = big.tile([P, MT], f32, tag="praw")
            qv = big.tile([P, MT], f32, tag="qv")       # -1000*positives
            gmin = big.tile([P, MT * NG], f32, tag="gmin")
            ssum = big.tile([P, MT * NG], f32, tag="ssum")
            t4 = big.tile([P, MT * NG], f32, tag="t4")
            st4 = big.tile([P, MT * NG], f32, tag="st4")
            mrow = big.tile([P, MT], f32, tag="mrow")
            stot = big.tile([P, MT], f32, tag="stot")
            lnst = big.tile([P, MT], f32, tag="lnst")
            outt = big.tile([P, 2 * MT], f32, tag="outt")
            cln1k = big.tile([P, 1], f32, tag="cln1k")
            nc.vector.memset(cln1k[:], LN_TEMP_INV)

            # ---------------- x-side prologue (own p1 shard)
            for m in range(MT):
                nc.sync.dma_start(xs[:, m * D:(m + 1) * D],
                                  p1[m * P:(m + 1) * P, :])
            for m in range(MT):
                sq = scr.tile([P, D], f32, tag="sq")
                nc.scalar.activation(sq[:], xs[:, m * D:(m + 1) * D],
                                     ACT.Square, accum_out=n2x[:, m:m + 1])
            # -1000/sqrt(s) = -exp(-0.5*ln(s) + ln(1000))
            nc.scalar.activation(tln[:, 0:MT], n2x[:], ACT.Ln)
            nc.scalar.activation(rix[:], tln[:, 0:MT], ACT.Exp, scale=-0.5,
                                 bias=cln1k[:])
            nc.vector.tensor_scalar_mul(rix[:], rix[:], -1.0)
            for m in range(MT):
                nc.vector.tensor_scalar(
                    out=xsb0[:, m, :], in0=xs[:, m * D:m * D + P],
                    scalar1=rix[:, m:m + 1], scalar2=None, op0=AOT.mult)
                nc.vector.tensor_scalar(
                    out=xsb1[:, m, :], in0=xs[:, m * D + P:(m + 1) * D],
                    scalar1=rix[:, m:m + 1], scalar2=None, op0=AOT.mult)
                nc.sync.dma_start_transpose(xT0[:, m * P:(m + 1) * P],
                                            xsb0[:, m, :])
                nc.sync.dma_start_transpose(xT1[:, m * P:(m + 1) * P],
                                            xsb1[:, m, :])

            # ---------------- own p2 shard: norms + positives (exact f32)
            for m in range(MT):
                nc.sync.dma_start(ys2[:, m * D:(m + 1) * D],
                                  p2s[m * P:(m + 1) * P, :])
            for m in range(MT):
                sq = scr.tile([P, D], f32, tag="sq")
                nc.scalar.activation(sq[:], ys2[:, m * D:(m + 1) * D],
                                     ACT.Square, accum_out=n2o[:, m:m + 1])
            nc.scalar.activation(tln[:, 0:MT], n2o[:], ACT.Ln)
            nc.scalar.activation(rio[:], tln[:, 0:MT], ACT.Exp, scale=-0.5)
            for m in range(MT):
                sq = scr.tile([P, D], f32, tag="sq")
                nc.gpsimd.tensor_tensor(sq[:], xs[:, m * D:(m + 1) * D],
                                        ys2[:, m * D:(m + 1) * D], AOT.mult)
                nc.vector.reduce_sum(out=praw[:, m:m + 1], in_=sq[:],
                                     axis=AXL.X)
            # q = praw * rix * rio = -1000 * positives
            nc.vector.tensor_mul(qv[:], praw[:], rix[:])
            nc.vector.tensor_mul(qv[:], qv[:], rio[:])

            # ---------------- main: y chunks pipelined with row-block mms
            xTk = (xT0, xT1)
            zTk = (z2T0, z2T1)
            with tc.tile_pool(name="mpsum", bufs=4, space="PSUM") as mpsum:
                for g in range(NG):
                    # y-chunk prep: 8 tiles -> normalized bf16 -> z2T cols
                    c0 = g * CHT
                    ytiles = []
                    for t in range(CHT):
                        yt = yin.tile([P, D], f32, tag="yt")
                        nc.sync.dma_start(
                            yt[:], p2[(c0 + t) * P:(c0 + t + 1) * P, :])
                        ytiles.append(yt)
                        # square+sum: gpsimd multiply, DVE free-axis reduce
                        sq = scr.tile([P, D], f32, tag="sq")
                        nc.gpsimd.tensor_tensor(sq[:], yt[:], yt[:], AOT.mult)
                        nc.vector.reduce_sum(
                            out=n2y[:, c0 + t:c0 + t + 1], in_=sq[:],
                            axis=AXL.X)
                    nc.scalar.activation(tln[:, c0:c0 + CHT],
                                         n2y[:, c0:c0 + CHT], ACT.Ln)
                    nc.scalar.activation(riy[:, c0:c0 + CHT],
                                         tln[:, c0:c0 + CHT],
                                         ACT.Exp, scale=-0.5)
                    for t in range(CHT):
                        yt = ytiles[t]
                        nc.gpsimd.tensor_scalar(
                            out=ysb0[:, t, :], in0=yt[:, 0:P],
                            scalar1=riy[:, c0 + t:c0 + t + 1],
                            scalar2=None, op0=AOT.mult)
                        nc.gpsimd.tensor_scalar(
                            out=ysb1[:, t, :], in0=yt[:, P:D],
                            scalar1=riy[:, c0 + t:c0 + t + 1],
                            scalar2=None, op0=AOT.mult)
                        nc.sync.dma_start_transpose(
                            z2T0[:, (c0 + t) * P:(c0 + t + 1) * P],
                            ysb0[:, t, :])
                        nc.sync.dma_start_transpose(
                            z2T1[:, (c0 + t) * P:(c0 + t + 1) * P],
                            ysb1[:, t, :])

                    # row-block matmuls + logsumexp pieces for this col group
                    for m in range(MT):
                        col = m * NG + g
                        pg = mpsum.tile([P, GROUP], f32, tag="pg")
                        for n in range(GROUP // NS):
                            for k in range(2):
                                nc.tensor.matmul(
                                    pg[:, n * NS:(n + 1) * NS],
                                    xTk[k][:, m * P:(m + 1) * P],
                                    zTk[k][:, g * GROUP + n * NS:
                                           g * GROUP + (n + 1) * NS],
                                    start=(k == 0), stop=(k == 1))
                        nc.vector.tensor_reduce(
                            out=gmin[:, col:col + 1], in_=pg[:],
                            axis=AXL.X, op=AOT.min)
                        # exp(1000*s - 1000*max) in place on PSUM, row-sums
                        nc.scalar.activation(pg[:], pg[:], ACT.Exp,
                                             scale=-1.0,
                                             bias=gmin[:, col:col + 1],
                                             accum_out=ssum[:, col:col + 1])

            # ---------------- exact fixup across groups, outputs
            for m in range(MT):
                c0, c1 = m * NG, (m + 1) * NG
                nc.vector.tensor_reduce(out=mrow[:, m:m + 1],
                                        in_=gmin[:, c0:c1],
                                        axis=AXL.X, op=AOT.min)
                nc.scalar.activation(t4[:, c0:c1], gmin[:, c0:c1],
                                     ACT.Exp, scale=-1.0,
                                     bias=mrow[:, m:m + 1])
                nc.vector.tensor_mul(st4[:, c0:c1], t4[:, c0:c1],
                                     ssum[:, c0:c1])
                nc.vector.reduce_sum(out=stot[:, m:m + 1], in_=st4[:, c0:c1],
                                     axis=AXL.X)
            nc.scalar.activation(lnst[:], stot[:], ACT.Ln)
            # loss rows = ln(stot) - mrow + q ;  q column = -1000*pos
            nc.vector.tensor_sub(outt[:, 0:MT], lnst[:], mrow[:])
            nc.vector.tensor_add(outt[:, 0:MT], outt[:, 0:MT], qv[:])
            nc.vector.tensor_copy(outt[:, MT:2 * MT], qv[:])
            nc.sync.dma_start(res[:, :], outt[:])

    nc.compile()
    return nc


def _get_nc():
    if "nc" not in _CACHE:
        _CACHE["nc"] = _build_nc()
    return _CACHE["nc"]


def run_cores(proj_1, proj_2, **spmd_kwargs):
    """Run the SPMD kernel; returns BassKernelResults."""
    from concourse.bass_utils import run_bass_kernel_spmd

    p1 = np.ascontiguousarray(np.asarray(proj_1, dtype=np.float32))
    p2 = np.ascontiguousarray(np.asarray(proj_2, dtype=np.float32))
    assert p1.shape == (B, D) and p2.shape == (B, D)
    in_maps = [
        {"p1": p1[c * R:(c + 1) * R], "p2": p2,
         "p2s": p2[c * R:(c + 1) * R]}
        for c in range(NCORES)
    ]
    nc = _get_nc()
    br = run_bass_kernel_spmd(nc, in_maps, core_ids=list(range(NCORES)),
                              **spmd_kwargs)
    return br


def kernel(proj_1, proj_2):
    br = run_cores(proj_1, proj_2)
    loss_sum = np.float64(0.0)
    q_sum = np.float64(0.0)
    for r in br.results:
        out = r["res"]
        loss_sum += np.float64(out[:, :MT].astype(np.float64).sum())
        q_sum += np.float64(out[:, MT:].astype(np.float64).sum())
    loss = np.float32(loss_sum / B)
    pos = np.float32(-q_sum / TEMP_INV)
    return (loss, pos)
